# revision 91
# baseline (speedup 1.0000x reference)
"""Group VQ (vq_codebook) Trainium2 Bass kernel.

Strategy: data-parallel over batch B=16 across 8 cores (2 batches/core).
Per core, per (group g, batch b, 125-token tile): two fp16 matmuls
[66,125]x[66,512] -> PSUM [125,1024] compute scores
s[t,k] = 2*x_t.e_k - |e_k|^2 (x rows 0..63 = fp16(x); rows 64,65 = 1.0;
E rows 64,65 carry -|e|^2 split hi/lo in fp16 so e2 is fp32-exact; the
single fp16 product term has ~6e-3 score noise, rescued on the host).

The 1024-code argmax is reduced on-device to 16 segment maxima per
tile (segment s = stride-16 residue class {s + 16*i}). Every f32 PSUM
score must be read out once by Act or DVE (walrus codegen rejects
GPSIMD tensor_tensor max / free-axis reduce / any PSUM operand, and
dma_start cannot read PSUM, so those are the only touch engines):
  T-tiles (10/32): one DVE tensor_reduce per tile straight from PSUM
      ([125,16,64] max over the stride-16 axis) - touch+reduce in one.
  S-tiles (22/32): Act stages PSUM->SBUF fp16 (Act has no reduce ops),
      then a binary fp16 max tree folds 1024->16: tensor_tensor max at
      the DVE 2x fast mode: one batched L1 fold per 4-tile quad, and
      slot-adjacent quad pairs fuse their deeper levels into t=8 fold
      chains (same elements, half the per-level access-latency inits);
      the 2-tile group sits between the Pool quad and the DVE quads.
The otherwise-idle GPSIMD (Pool) engine folds group 0's whole tree and
pair 2's L1 using the 3-op emulation max(a,b) = b + relu(a-b)
(tensor_sub + tensor_scalar_max + tensor_add, all legal on Pool fp16
SBUF; sub/add model at 0.42 Q7 efficiency) - ~9x a DVE fold per
element, but it relieves the real bottleneck. Modeled busy per (g,b):
Act ~22.9us (22 stagings), DVE ~23.4us (10 TRs + 5 trees), Pool
~23.3us (group 0's emulated tree), PE ~13.8us; ~379us/core end to end
vs the 476.8us all-Act-stage + all-DVE-tree baseline. Cold start is
hidden by ~2us of throwaway matmuls that ramp the PE p-state while the
first DMAs land; the last (g,b) runs Pool-free so its lagged stream
never closes the program.
PSUM holds two independent double-buffered 2-bank tile pools (one per
touch engine) so Act and DVE drain concurrently; S/T tiles interleave
so neither pool's WAR stalls block the in-order PE stream. Tree chains
are stream-aligned with their engine (tails never park on the other
engine's L1, which would block ready work behind them in the in-order
queues); the one cross edge (Pool L1 -> DVE tail of group 1) is issued
4 tiles late. Codebook tiles load lazily (group g+1 prefetches behind
group g's first x DMA); the cold-start x DMA is split so the first
matmuls aren't queued behind the full 264KB.

Host: picks the top-2 segments per token from the device maxima and
rescores their 2x64 candidate codes exactly in fp32 via per-(group,
segment) batched sgemm, then gathers code vectors. The top-2 rescue
makes fp16 score/seg-max noise (incl. the ~1-ulp emulated-max error)
harmless: a wrong final pick needs a 3-way cross-segment near-tie, so
end-to-end mismatches stay at the fp32 reference's own near-tie level
(rel ~2e-3, threshold 2e-2).
The device emits tile maxima in schedule order (S-tiles first, then
T-tiles); _SLOT_OF maps token-tile -> om slot and the host inverts it.
"""
import sys
import numpy as np
from contextlib import ExitStack

sys.path.insert(0, "/opt/trn_rl_repo")

B, C, F, T = 16, 2, 256, 4000
G, K, D = 8, 1024, 64
NCORES = 8
NB = B // NCORES          # batches per core = 2
TT = 125                  # tokens per tile (4000 = 32*125)
ST = 2000                 # tokens per x-DMA supertile (16 tiles)
NTILES = T // TT          # 32
NSUP = T // ST            # 2
TPS = ST // TT            # tiles per supertile = 16
NSEG = 16                 # segments per 1024 codes
SEGW = K // NSEG          # 64 candidate codes per segment (stride-16)
XR = D + 2                # x rows: 64 features + two ones rows = 66

USE_POOL = True
POOL_TREE_GROUPS = {0} if USE_POOL else set()


def _interleave(n_s, n_t):
    seq, a, b = [], 0, 0
    for _ in range(n_s + n_t):
        if (a * n_t <= b * n_s and a < n_s) or b == n_t:
            seq.append("S")
            a += 1
        else:
            seq.append("T")
            b += 1
    return seq


def _mk_sched(groups, st_split):
    """Per-(g,b) schedule: S/T order, om slot of each token-tile
    (staged tiles take slots 0..NS-1 in arrival order so tree groups
    write contiguous slot ranges; TR tiles NS..31), and group lookup
    tables. Supertile 1's pattern is reversed so the (g,b) boundary
    region is staged-tile-heavy: trailing TR tiles would park the next
    (g,b)'s first matmuls behind ppT WARs in the in-order PE stream."""
    ns = sum(n for _, n in groups)
    seq = _interleave(*st_split[0]) + _interleave(*st_split[1])[::-1]
    assert seq.count("S") == ns
    slot_of = np.empty(NTILES, dtype=np.int64)
    s_idx_of = {}
    s = t = 0
    for k, c in enumerate(seq):
        if c == "S":
            slot_of[k] = s
            s_idx_of[k] = s
            s += 1
        else:
            slot_of[k] = ns + t
            t += 1
    grp_of = {}
    for gi, (g0, n) in enumerate(groups):
        for j in range(g0, g0 + n):
            grp_of[j] = gi
    return dict(groups=groups, ns=ns, seq=seq, slot_of=slot_of,
                s_idx_of=s_idx_of, grp_of=grp_of,
                pos_of_sidx=[k for k, c in enumerate(seq) if c == "S"])


# variant A: 22 staged (5 quads + a mid-schedule pair so the (g,b)
# doesn't end on a small straggler tail) / 10 TR tiles.
# variant B: 24 staged (6 quads) / 8 TR tiles - shifts ~1.3us of DVE
# work per (g,b) onto Act's slack; used where DVE is tightest.
_SCHED_A = _mk_sched(((0, 4), (4, 2), (6, 4), (10, 4), (14, 4), (18, 4)),
                     ((11, 5), (11, 5)))
_SCHED_B = _mk_sched(((0, 4), (4, 4), (8, 4), (12, 4), (16, 4), (20, 4)),
                     ((12, 4), (12, 4)))
_B_GBS = () if USE_POOL else tuple(range(16))
_GB_SCHED = [_SCHED_B if gb in _B_GBS else _SCHED_A for gb in range(16)]
# host-side om slot permutation per (g, local-batch)
_SLOTS = np.stack([s["slot_of"] for s in _GB_SCHED]).reshape(G, NB, NTILES)

_compiled = None


def _build_program():
    import concourse.bass as bass
    import concourse.tile as tile
    from concourse import bacc, mybir

    nc = bacc.Bacc(
        "TRN2",
        target_bir_lowering=False,
        debug=False,
        enable_asserts=False,
        num_devices=NCORES,
    )
    f32 = mybir.dt.float32
    f16 = mybir.dt.float16
    xa = nc.dram_tensor("xa", [NB, G, XR, T], f16, kind="ExternalInput").ap()
    et = nc.dram_tensor("et", [G, XR, K], f16, kind="ExternalInput").ap()
    om = nc.dram_tensor(
        "om", [G * NB, TT, NTILES * NSEG], f16, kind="ExternalOutput"
    ).ap()

    def views(out_flat, in_flat, t, w):
        ri = in_flat.rearrange("p (t k) -> p t k", t=t, k=w)
        ro = out_flat.rearrange("p (t k) -> p t k", t=t, k=w // 2)
        return ro, ri[:, :, :w // 2], ri[:, :, w // 2:]

    def fold(out_flat, in_flat, t, w):
        """Elementwise fp16 max of the two halves of each w-wide block:
        in [p, t*w] -> out [p, t*(w/2)]; pairs element j with j+w/2."""
        ro, lo, hi = views(out_flat, in_flat, t, w)
        nc.vector.tensor_max(ro, lo, hi)

    with tile.TileContext(nc) as tc, ExitStack() as ctx:
        epool = ctx.enter_context(tc.tile_pool(name="e", bufs=1))
        xpool = ctx.enter_context(tc.tile_pool(name="x", bufs=4))
        # 2-bank PSUM tiles ([125, 1024] f32) in two independent
        # double-buffered pools, one per touch engine: Act drains ppA
        # while DVE drains ppT, PE filling each pool's sibling buffer. A
        # single shared pool would recycle in issue order and serialize
        # the touch engines behind each other.
        ppA = ctx.enter_context(
            tc.tile_pool(name="psA", bufs=2, space=bass.MemorySpace.PSUM)
        )
        ppT = ctx.enter_context(
            tc.tile_pool(name="psT", bufs=2, space=bass.MemorySpace.PSUM)
        )
        # 6 staged-tile buffers: 5 quad-groups per (g,b) share the ring,
        # and Pool's batched L1 holds group 0's tile for ~4us after the
        # last staging - at bufs=4 the 4th quad's staging parks on that
        # WAR for ~3us every (g,b)
        spool = ctx.enter_context(tc.tile_pool(name="scp", bufs=7))
        tpoolD = ctx.enter_context(tc.tile_pool(name="treeD", bufs=2))
        tpoolP = ctx.enter_context(tc.tile_pool(name="treeP", bufs=2))
        mpool = ctx.enter_context(tc.tile_pool(name="mseg", bufs=3))

        def fold_emu(out_flat, in_flat, t, w, tmp):
            """Pool 3-op fold: max(a,b) = b + relu(a-b)."""
            ro, lo, hi = views(out_flat, in_flat, t, w)
            tv = tmp[:, :t * (w // 2)].rearrange(
                "p (t k) -> p t k", t=t, k=w // 2)
            nc.gpsimd.tensor_sub(tv, lo, hi)
            nc.gpsimd.tensor_scalar_max(tv, tv, 0.0)
            nc.gpsimd.tensor_add(ro, tv, hi)

        etiles = {}

        def load_e(gi):
            if gi not in etiles:
                e_t = epool.tile([XR, K], f16, tag=f"e{gi}", name=f"et{gi}")
                nc.sync.dma_start(e_t[:], et[gi])
                etiles[gi] = e_t
            return etiles[gi]

        # only group 0's codebook ahead of the first x tile; the rest
        # prefetch behind each group's first x DMA (overlapped with compute)
        load_e(0)

        # x supertile DMAs are hoisted one (g,b) ahead: an om DMA's
        # dependency waits hold the SP sequencer, so anything issued
        # after it fires only once the previous (g,b) fully reduces -
        # issuing x(gb+1) mid-gb keeps the next (g,b)'s matmuls fed.
        xtiles = {}

        def issue_x(gb2, sup2, split=False):
            g2, b2 = divmod(gb2, NB)
            xt = xpool.tile([XR, ST], f16, tag="x", name="xt")
            if split:
                # split the cold-start DMA so the first matmuls aren't
                # queued behind all 264KB
                nc.sync.dma_start(xt[:, :4 * TT], xa[b2, g2, :, :4 * TT])
                nc.sync.dma_start(
                    xt[:, 4 * TT:], xa[b2, g2, :, 4 * TT:ST])
            else:
                nc.sync.dma_start(
                    xt[:],
                    xa[b2, g2, :, sup2 * ST:(sup2 + 1) * ST])
            xtiles[(gb2, sup2)] = xt

        issue_x(0, 0, split=True)
        issue_x(0, 1)

        # PE p-state warmup: ~2us of throwaway matmuls (into the first
        # ppA ring slot, from a never-written SBUF tile) while the first
        # x/e DMAs land, so the real matmuls start at the max p-state
        # instead of paying the 2.4x cold-clock tax
        warm_in = spool.tile([XR, 256], f16, tag="warm", name="warm")
        nc.gpsimd.memset(warm_in[:], 0.0)
        warm_ps = ppA.tile([TT, K], f32, tag="ps", name="ps")
        for wi in range(8):
            nc.tensor.matmul(
                warm_ps[:, wi % 4 * 256:(wi % 4 + 1) * 256],
                warm_in[:, :TT], warm_in[:],
                start=True, stop=True)

        for g in range(G):
            for b in range(NB):
                gb = g * NB + b
                sched = _GB_SCHED[gb]
                GROUPS = sched["groups"]
                # the final (g,b)'s trees all go to DVE: Pool's stream
                # (start lag + busy + per-op bubbles) would otherwise
                # close the program well after everyone else
                last = gb == G * NB - 1
                pool_l1_groups = (set(POOL_TREE_GROUPS) if not last
                                  else set())
                pool_tail_groups = (set(POOL_TREE_GROUPS) if not last
                                    else set())
                # tail "units": slot-adjacent DVE quads pair into one
                # t=8 fold chain (same elements, half the per-level
                # access-latency inits); Pool's quad and the 2-tile
                # group stay solo. The Pool-free last (g,b) fuses its
                # quad 0 with the pair group as a t=6 unit.
                if not last:
                    units = ((0,), (1,), (2, 3), (4, 5))
                else:
                    # groups 4 and 5 close as four independent
                    # pair-trees (existing d2 tile tags, shared d8 L1
                    # tile): all but the last are hidden under Act's
                    # remaining stagings, so only ~1.5us follows the
                    # final staging
                    units = ((0, 1), (2, 3), (4, 5))
                unit_of = {g2: u for u in units for g2 in u}
                offs = {}
                for u in units:
                    o = 0
                    for g2 in u:
                        offs[g2] = o
                        o += GROUPS[g2][1]

                m_sb = mpool.tile([TT, NTILES * NSEG], f16)
                scps = {}   # group id -> staged fp16 tile
                l1s = {}    # group id -> L1 output tile
                tmps = {}   # group id -> Pool emu scratch tile
                deferred = {}  # seq position -> [callable]

                def tail(u):
                    gi = GROUPS[u[0]][0]
                    n = sum(GROUPS[g2][1] for g2 in u)
                    on_pool = u[0] in pool_tail_groups and len(u) == 1
                    tp = tpoolP if on_pool else tpoolD
                    tg = ("p" if on_pool else "d") + str(n)
                    l1 = l1s.pop(u)
                    steps = []
                    w = 512
                    cur = l1
                    while w > 32:
                        nxt = tp.tile([TT, n * (w // 2)], f16,
                                      tag=f"{tg}w{w}", name=f"{tg}w{w}")
                        steps.append((nxt, cur, w))
                        cur = nxt
                        w //= 2
                    for nxt, src, wi in steps:
                        if on_pool:
                            fold_emu(nxt[:], src[:], n, wi, tmps[u[0]])
                        else:
                            fold(nxt[:], src[:], n, wi)
                    if on_pool:
                        fold_emu(m_sb[:, gi * NSEG:(gi + n) * NSEG],
                                 cur[:], n, 32, tmps.pop(u[0]))
                    else:
                        fold(m_sb[:, gi * NSEG:(gi + n) * NSEG],
                             cur[:], n, 32)

                def pair_tail(l1_ap, gi):
                    cur = l1_ap
                    w = 512
                    while w > 32:
                        nxt = tpoolD.tile([TT, w], f16,
                                          tag=f"d2w{w}", name=f"d2w{w}")
                        fold(nxt[:], cur, 2, w)
                        cur = nxt[:]
                        w //= 2
                    fold(m_sb[:, gi * NSEG:(gi + 2) * NSEG], cur, 2, 32)

                for k in range(NTILES):
                    sup, ki = divmod(k, TPS)
                    if k == 0 and b == 0 and g + 1 < G:
                        load_e(g + 1)
                    if k == TPS and gb + 1 < G * NB:
                        issue_x(gb + 1, 0)
                    if k == TPS + 8 and gb + 1 < G * NB:
                        issue_x(gb + 1, 1)
                    xt = xtiles[(gb, sup)]
                    pool = ppA if sched["seq"][k] == "S" else ppT
                    ps = pool.tile([TT, K], f32, tag="ps", name="ps")
                    xsl = xt[:, ki * TT:(ki + 1) * TT]
                    nc.tensor.matmul(
                        ps[:, :K // 2], xsl,
                        etiles[g][:, :K // 2], start=True, stop=True)
                    nc.tensor.matmul(
                        ps[:, K // 2:], xsl,
                        etiles[g][:, K // 2:], start=True, stop=True)
                    slot = int(sched["slot_of"][k])
                    if sched["seq"][k] == "T":
                        nc.vector.tensor_reduce(
                            m_sb[:, slot * NSEG:(slot + 1) * NSEG],
                            ps[:].rearrange(
                                "p (w s) -> p s w", w=SEGW, s=NSEG),
                            axis=mybir.AxisListType.X,
                            op=mybir.AluOpType.max,
                        )
                    else:
                        sidx = sched["s_idx_of"][k]
                        grp = sched["grp_of"][sidx]
                        gi, n = GROUPS[grp]
                        pos = sidx - gi
                        if pos == 0:
                            scps[grp] = spool.tile([TT, n * K], f16,
                                                   tag=f"scp{n}",
                                                   name=f"scp{n}")
                            on_pool = grp in pool_l1_groups
                            u = unit_of[grp]
                            if u not in l1s:
                                nu = sum(GROUPS[g2][1] for g2 in u)
                                tp = tpoolP if on_pool else tpoolD
                                tg = ("p" if on_pool else "d") + str(nu)
                                l1s[u] = tp.tile([TT, nu * 512], f16,
                                                 tag=tg + "l1",
                                                 name=tg + "l1")
                            if on_pool:
                                tmps[grp] = tpoolP.tile(
                                    [TT, n * 512], f16,
                                    tag="ptmp", name="ptmp")
                        nc.scalar.activation(
                            scps[grp][:, pos * K:(pos + 1) * K], ps[:],
                            mybir.ActivationFunctionType.Copy)
                        if grp in pool_l1_groups:
                            # Pool L1 batched once per group (a third
                            # fewer Q7 launches and bubbles), except the
                            # cold-start group, where per-pair L1s let
                            # Pool begin ~2 tiles earlier
                            if gb == 0 and pos % 2 == 1:
                                fold_emu(
                                    l1s[unit_of[grp]][:, (pos - 1) * 512:
                                                      (pos + 1) * 512],
                                    scps[grp][:, (pos - 1) * K:
                                              (pos + 1) * K],
                                    2, 1024, tmps[grp])
                            elif gb > 0 and pos == n - 1:
                                fold_emu(l1s[unit_of[grp]][:],
                                         scps[grp][:],
                                         n, 1024, tmps[grp])
                        elif last and grp in (4, 5):
                            # the closing groups fold L1 and tail per
                            # pair so only the final pair's ~1.5us chain
                            # is exposed after the last staging
                            if pos % 2 == 1:
                                u = unit_of[grp]
                                off = (offs[grp] + pos - 1) * 512
                                fold(l1s[u][:, off:off + 1024],
                                     scps[grp][:,
                                               (pos - 1) * K:
                                               (pos + 1) * K],
                                     2, 1024)
                                gi_p = GROUPS[grp][0] + pos - 1
                                lag = 1 if (grp == 5 and pos == 3) else 2
                                due = min(
                                    sched["pos_of_sidx"][sidx] + lag,
                                    NTILES - 1)
                                deferred.setdefault(due, []).append(
                                    lambda l1=l1s[u], off=off,
                                    gi_p=gi_p: pair_tail(
                                        l1[:, off:off + 1024], gi_p))
                        elif pos == n - 1:
                            # DVE L1 batched once per group: DVE is
                            # saturated mid-stream, so one instruction's
                            # access-latency init beats an early start
                            u = unit_of[grp]
                            off = offs[grp] * 512
                            fold(l1s[u][:, off:off + n * 512],
                                 scps[grp][:], n, 1024)
                        if pos == n - 1:
                            scps.pop(grp)
                            u = unit_of[grp]
                            if last and grp in (4, 5):
                                pass  # pair-tails scheduled at each L1
                            elif grp == u[-1]:
                                # defer tails so a just-issued L1 dep
                                # never parks them at a queue head
                                due = min(sched["pos_of_sidx"][sidx] + 3,
                                          NTILES - 1)
                                deferred.setdefault(due, []).append(
                                    lambda u=u: tail(u))
                    for fn in deferred.pop(k, ()):
                        fn()
                for posn in sorted(deferred):
                    for fn in deferred[posn]:
                        fn()
                nc.sync.dma_start(om[gb], m_sb[:])

    nc.compile()
    return nc


def _get_compiled():
    global _compiled
    if _compiled is None:
        _compiled = _build_program()
    return _compiled


def _prep_inputs(x, codebooks):
    # xa: [B, G, 66, T] fp16 — rows 0..63 = fp16(x), rows 64,65 = 1.0
    xg = x.reshape(B, G, D, T)
    xa = np.empty((B, G, XR, T), dtype=np.float16)
    xa[:, :, :D] = xg
    xa[:, :, D:] = 1.0
    # et: [G, 66, K] fp16 — rows 0..63 = 2*E^T; rows 64,65 = -|e|^2 hi/lo
    e2 = (codebooks.astype(np.float32) ** 2).sum(-1)          # [G, K]
    eh = (-e2).astype(np.float16)
    el = (-e2 - eh.astype(np.float32)).astype(np.float16)
    et = np.empty((G, XR, K), dtype=np.float16)
    et[:, :D] = 2.0 * np.transpose(codebooks, (0, 2, 1))
    et[:, D] = eh
    et[:, D + 1] = el
    return xa, et


def run_device(x, codebooks, trace=False):
    from concourse.bass_utils import run_bass_kernel_spmd

    nc = _get_compiled()
    xa, et = _prep_inputs(np.asarray(x, np.float32),
                          np.asarray(codebooks, np.float32))
    in_maps = []
    for core in range(NCORES):
        sl = slice(core * NB, (core + 1) * NB)
        in_maps.append({"xa": np.ascontiguousarray(xa[sl]), "et": et})
    res = run_bass_kernel_spmd(nc, in_maps, list(range(NCORES)), trace=trace)
    return res


# candidate code indices per segment: stride-16 residue classes [NSEG, SEGW]
_CAND = np.arange(NSEG)[:, None] + NSEG * np.arange(SEGW)[None, :]


def _host_finish(x, codebooks, m16):
    """m16: [G, B, T, NSEG] fp32 device segment maxima.
    Rescore the top-2 segments' 2*64 candidates exactly in fp32."""
    xg = x.reshape(B, G, D, T)
    # tokens as [G, B*T, D]
    tok = np.ascontiguousarray(
        np.transpose(xg, (1, 0, 3, 2)).reshape(G, B * T, D))
    m2 = m16.reshape(G, B * T, NSEG)
    # top-2 segments per token
    s1 = np.argmax(m2, axis=-1)                               # [G, N]
    m2m = np.copy(m2)
    np.put_along_axis(m2m, s1[..., None], -np.inf, axis=-1)
    s2 = np.argmax(m2m, axis=-1)                              # [G, N]
    out = np.empty((B, G, D, T), dtype=np.float32)
    n = B * T
    for g in range(G):
        cb = codebooks[g].astype(np.float32)                  # [K, D]
        e2 = (cb * cb).sum(-1)                                # [K]
        w = 2.0 * cb.T                                        # [D, K]
        best_val = np.full(n, -np.inf, dtype=np.float32)
        best_idx = np.zeros(n, dtype=np.int64)
        for seg in range(NSEG):
            cand = _CAND[seg]                                 # [64]
            mask = (s1[g] == seg) | (s2[g] == seg)
            rows = np.nonzero(mask)[0]
            if rows.size == 0:
                continue
            a = tok[g][rows]                                  # [N_s, D]
            sc = a @ w[:, cand] - e2[cand]                    # [N_s, 64]
            loc = np.argmax(sc, axis=1)
            val = sc[np.arange(rows.size), loc]
            idx = cand[loc]
            upd = (val > best_val[rows]) | (
                (val == best_val[rows]) & (idx < best_idx[rows]))
            r_upd = rows[upd]
            best_val[r_upd] = val[upd]
            best_idx[r_upd] = idx[upd]
        q = cb[best_idx]                                      # [N, D]
        out[:, g] = q.reshape(B, T, D).transpose(0, 2, 1)
    return out.reshape(B, C, F, T)


def kernel(x, codebooks):
    x = np.asarray(x, dtype=np.float32)
    codebooks = np.asarray(codebooks, dtype=np.float32)
    res = run_device(x, codebooks)
    # om [G*NB, TT, NTILES*NSEG] fp16 in schedule-slot order; token-tile k
    # lives at slot _SLOT_OF[k]; token t = k*TT + p
    m16 = np.empty((G, B, T, NSEG), dtype=np.float32)
    for core in range(NCORES):
        o = res.results[core]["om"].astype(np.float32).reshape(
            G, NB, TT, NTILES, NSEG)
        o = np.take_along_axis(o, _SLOTS[:, :, None, :, None], axis=3)
        m16[:, core * NB:(core + 1) * NB] = o.transpose(0, 1, 3, 2, 4).reshape(
            G, NB, T, NSEG
        )
    q = _host_finish(x, codebooks, m16)
    x_q = x + (q - x)
    return x_q, q
